# revision 8
# baseline (speedup 1.0000x reference)
"""Trainium2 Bass kernel for nn_Loss_20933670601009 (gathered-prob NLL loss).

Sharding: data-parallel over L_a (16 rows x 8 cores, 512 positions/core);
host sums the 8 per-core scalars (with on-device -1/32 scaling this equals
mean-over-batch of per-sequence sums).  ~36.5us baseline -> ~28-29us.

From HW traces:
  - Q7/SWDGE emission (~1.1us/instruction, serial on the Pool engine) is the
    core cost; ref (512 entries/position) moves to a dense HWDGE-streamed
    one-hot multiply + block reduce on DVE, hidden under the rule+token
    emission window.  8 gathers instead of 12.
  - CCE-accumulate gathers emit two descriptor blocks (~1.77us each) — worse
    than plain gathers + two DVE adds.  Plain bypass gathers + zero-pad:
    gt==-1 offsets point at a host-appended 0.0 element, so no validity
    masking is needed.
  - ref data + one-hot go as bf16 (1MB total, not 4MB): the f32 flood
    starved the meta DMA's completion semaphore and delayed the first
    gather by ~5us.  With one nonzero per 512-block the reduce is exact
    selection; only the selected ref value rounds to bf16 (~0.4% of one of
    three summands, far under the 2e-2 gate).
  - meta loads via SWDGE from Q7 itself (first post-barrier instruction),
    dodging HWDGE-ring congestion; a dummy Ln activation hoists the ACT
    table load out of the gather window.
"""

import os
import sys

import numpy as np

for _p in ("/opt/trn_rl_repo", "/root/.axon_site/_ro/trn_rl_repo"):
    if os.path.isdir(_p) and _p not in sys.path:
        sys.path.insert(0, _p)

L_A, B = 128, 32
V_RULE, V_TOK, V_REF = 2048, 32000, 512
EPS = 1e-07
N_CORES = 8
L_SH = L_A // N_CORES            # 16 sequence rows per core
NPOS = L_SH * B                  # 512 positions per core
P = 128                          # SBUF partitions
J = NPOS // P                    # 4 positions per partition
SEG = (0, NPOS * V_RULE)         # rule, token segment bases in flat
VS = (V_RULE, V_TOK)
N_FLAT = NPOS * (V_RULE + V_TOK) + 2   # +2 pads: [N-2]=0.0, [N-1]=1.0

_CACHE = {}


def _build():
    import concourse.bacc as bacc
    import concourse.bass as bass
    import concourse.mybir as mybir
    import concourse.tile as tile

    f32 = mybir.dt.float32
    bf16 = mybir.dt.bfloat16
    i32 = mybir.dt.int32

    nc = bacc.Bacc(
        "TRN2",
        target_bir_lowering=False,
        debug=False,
        enable_asserts=False,
        num_devices=N_CORES,
    )

    # meta layout (int32 [128, 8]): absolute flat gather offsets
    # (rule 4 | token 4); host folds in segment base, row base, the gt==-1
    # clamp (-> 0.0 pad) and the loss mask (mask==0 -> rule reads the 1.0
    # pad, token the 0.0 pad, so s=1 and ln(s+eps)~0 with no masking ops)
    meta_d = nc.dram_tensor("meta", [P, 8], i32, kind="ExternalInput").ap()
    flat_d = nc.dram_tensor("probs_flat", [N_FLAT, 1], f32, kind="ExternalInput").ap()
    refd_d = nc.dram_tensor("ref_data", [P, J, V_REF], bf16, kind="ExternalInput").ap()
    refoh_d = nc.dram_tensor("ref_oh", [P, J, V_REF], bf16, kind="ExternalInput").ap()
    out_d = nc.dram_tensor("out", [1, 1], f32, kind="ExternalOutput").ap()

    with tile.TileContext(nc) as tc:
        with (
            tc.tile_pool(name="sb", bufs=1) as pool,
            tc.tile_pool(name="ps", bufs=1, space="PSUM") as psum,
        ):
            # meta via HWDGE, first SP instruction; the 1MB ref flood is
            # gated behind meta's completion so it cannot delay meta's
            # semaphore receipt (observed +3-4us otherwise)
            gate = nc.alloc_semaphore(name="refgate")
            meta = pool.tile([P, 8], i32)
            nc.sync.dma_start(out=meta[:], in_=meta_d[:])
            offs = meta[:, 0:8]

            epst = pool.tile([P, 1], f32)
            nc.gpsimd.memset(epst[:], EPS)

            # hoist the Ln ACT-table load out of the gather window
            dact = pool.tile([1, 1], f32)
            nc.gpsimd.memset(dact[:], 1.0)
            dact2 = pool.tile([1, 1], f32)
            nc.scalar.activation(
                out=dact2[:], in_=dact[:], func=mybir.ActivationFunctionType.Ln
            )

            # DVE probe: idempotent self-write on a TOKEN offset cell —
            # auto-waits meta's DMA sem, stays live via the gather reads,
            # then opens the gate.  DVE int ops are f32-backed; token offsets
            # (< 2^24) and the even 0.0-pad index survive the round-trip
            # exactly, while rule cells can hold the odd 1.0-pad index which
            # would be corrupted.
            nc.vector.tensor_scalar(
                out=meta[0:1, 4:5], in0=meta[0:1, 4:5], scalar1=0, scalar2=None,
                op0=mybir.AluOpType.max,
            )
            nc.vector.sem_inc(gate, 1)

            # dense ref path: stream data + one-hot on the two HWDGE rings
            rdt = pool.tile([P, J, V_REF], bf16)
            roh = pool.tile([P, J, V_REF], bf16)
            nc.scalar.wait_ge(gate, 1)
            nc.scalar.dma_start(out=rdt[:], in_=refd_d[:])
            nc.sync.wait_ge(gate, 1)
            nc.sync.dma_start(out=roh[:], in_=refoh_d[:])

            # 8 element-gathers (one offset per partition row, 1 f32 each)
            gm = pool.tile([P, 8], f32)
            for col in range(8):
                nc.gpsimd.indirect_dma_start(
                    out=gm[:, col:col + 1],
                    out_offset=None,
                    in_=flat_d[:],
                    in_offset=bass.IndirectOffsetOnAxis(
                        ap=offs[:, col:col + 1], axis=0
                    ),
                    element_offset=0,
                )

            # ref select: one-hot multiply + per-position 512-block reduce
            # (one-hot rows are all-zero for gt_ref == -1)
            rm = pool.tile([P, J, V_REF], bf16)
            nc.vector.tensor_mul(out=rm[:], in0=rdt[:], in1=roh[:])
            gref = pool.tile([P, J, 1], f32)
            nc.vector.reduce_sum(out=gref[:], in_=rm[:], axis=mybir.AxisListType.X)

            # first add needs only rule gathers (1-4) + the ref path, so it
            # completes inside the token-gather window; only the second add
            # trails the final gather's completion semaphore
            s1 = pool.tile([P, J], f32)
            nc.vector.tensor_add(out=s1[:], in0=gm[:, 0:4], in1=gref[:, :, 0])
            s = pool.tile([P, J], f32)
            nc.vector.tensor_add(out=s[:], in0=s1[:], in1=gm[:, 4:8])

            # ln(s + eps) with the per-partition row sum accumulated in the
            # same ACT instruction.  bias=eps matches the reference exactly
            # for s < eps and is within eps/s elsewhere (never material for
            # sums of uniforms); masked positions arrive as s=1 -> ~0.
            ln = pool.tile([P, J], f32)
            rs = pool.tile([P, 1], f32)
            nc.scalar.activation(
                out=ln[:], in_=s[:], func=mybir.ActivationFunctionType.Ln,
                bias=epst[:], accum_out=rs[:],
            )

            # partition reduction via PE; weight -1/B folds negation + mean
            negw = pool.tile([P, 1], f32)
            nc.gpsimd.memset(negw[:], -1.0 / B)
            acc = psum.tile([1, 1], f32)
            nc.tensor.matmul(out=acc[:], lhsT=rs[:], rhs=negw[:], start=True, stop=True)
            res = pool.tile([1, 1], f32)
            nc.scalar.copy(out=res[:], in_=acc[:])
            nc.sync.dma_start(out=out_d[:], in_=res[:])

    nc.compile()
    return nc


def get_nc():
    if "nc" not in _CACHE:
        _CACHE["nc"] = _build()
    return _CACHE["nc"]


def make_in_maps(rule_probs, token_probs, reference_probs, ground_truth_actions, mask):
    """Shard the full inputs into 8 per-core input maps."""
    import ml_dtypes

    rule_probs = np.ascontiguousarray(np.asarray(rule_probs, dtype=np.float32))
    token_probs = np.ascontiguousarray(np.asarray(token_probs, dtype=np.float32))
    reference_probs = np.ascontiguousarray(np.asarray(reference_probs, dtype=np.float32))
    gt = np.asarray(ground_truth_actions, dtype=np.int32)
    mask = np.asarray(mask, dtype=np.int32)

    q = np.arange(NPOS, dtype=np.int64)
    pad0 = N_FLAT - 2  # appended 0.0
    pad1 = N_FLAT - 1  # appended 1.0

    in_maps = []
    for i in range(N_CORES):
        lo, hi = i * L_SH, (i + 1) * L_SH
        gt_sh = gt[lo:hi].reshape(NPOS, 3)
        msk = mask[lo:hi].reshape(NPOS) != 0
        meta = np.empty((P, 8), np.int32)
        for c in range(2):
            idx = gt_sh[:, c].astype(np.int64)
            off = np.where(idx >= 0, SEG[c] + q * VS[c] + np.maximum(idx, 0), pad0)
            if c == 0:
                off = np.where(msk, off, pad1)  # masked: rule reads 1.0
            else:
                off = np.where(msk, off, pad0)  # masked: token reads 0.0
            meta[:, c * 4:(c + 1) * 4] = off.reshape(P, J).astype(np.int32)
        probs_flat = np.concatenate(
            [
                rule_probs[lo:hi].reshape(-1),
                token_probs[lo:hi].reshape(-1),
                np.array([0.0, 1.0], np.float32),
            ]
        )
        idx_ref = gt_sh[:, 2]
        oh = np.zeros((NPOS, V_REF), np.float32)
        valid = (idx_ref >= 0) & msk
        oh[np.arange(NPOS)[valid], idx_ref[valid]] = 1.0
        in_maps.append(
            {
                "meta": meta,
                "probs_flat": probs_flat.reshape(-1, 1),
                "ref_data": reference_probs[lo:hi]
                .reshape(P, J, V_REF)
                .astype(ml_dtypes.bfloat16),
                "ref_oh": oh.reshape(P, J, V_REF).astype(ml_dtypes.bfloat16),
            }
        )
    return in_maps


def run(inputs, trace=False, trace_cores=None):
    from concourse.bass_utils import run_bass_kernel_spmd

    nc = get_nc()
    in_maps = make_in_maps(**inputs)
    res = run_bass_kernel_spmd(
        nc,
        in_maps,
        core_ids=list(range(N_CORES)),
        trace=trace,
        trace_cores=trace_cores,
    )
    total = np.float64(0.0)
    for r in res.results:
        total += np.float64(r["out"].reshape(())[()])
    return np.asarray(total, dtype=np.float32), res


def kernel(**inputs) -> np.ndarray:
    out, _ = run(inputs)
    if not np.isfinite(out):
        # rare device-side first-run flake (seen once: NaN from a fresh NEFF
        # load); one retry is strictly better than returning garbage
        out, _ = run(inputs)
    return out
